# revision 20
# baseline (speedup 1.0000x reference)
"""GQA attention layer (QKV proj + RoPE + softmax attention + out proj) on 8
Trainium2 NeuronCores.

Sharding: core c = (batch b, head-group g) with b = c // 4, g = c % 4.
Each core handles one batch and one GQA group (4 q heads + 1 kv head),
computes a w_o-column-sliced partial output (row-parallel), and the host sums
the 4 partials per batch (the "all-reduce" of the hinted TP scheme, done on
host since the final gather happens there anyway).
"""

import numpy as np
import ml_dtypes

B, S, HID = 2, 2048, 1024
NH, NKV, D = 16, 4, 64
SCALE = D ** -0.5
NCORES = 8
TT = 512          # token tile (projection N / q tile)
NTT = S // TT     # 4
KC = S // 128     # 16 k chunks

_BF16 = ml_dtypes.bfloat16

_nc_cache = None


def _build_bass():
    import concourse.bass as bass
    import concourse.mybir as mybir
    import concourse.tile as tile
    from concourse import bacc
    from concourse.masks import make_identity

    BF = mybir.dt.bfloat16
    F32 = mybir.dt.float32
    AF = mybir.ActivationFunctionType
    MULT = mybir.AluOpType.mult
    ADD = mybir.AluOpType.add

    nc = bacc.Bacc()
    hT = nc.dram_tensor("hT", (HID, S), BF, kind="ExternalInput")
    wqkT = nc.dram_tensor("wqkT", (HID, 384), BF, kind="ExternalInput")
    woT = nc.dram_tensor("woT", (256, HID), BF, kind="ExternalInput")
    cosd = nc.dram_tensor("cosd", (64, S), F32, kind="ExternalInput")
    sind = nc.dram_tensor("sind", (64, S), F32, kind="ExternalInput")
    out = nc.dram_tensor("out", (S, HID), F32, kind="ExternalOutput")

    with tile.TileContext(nc) as tc:
        with (
            tc.tile_pool(name="persist", bufs=1) as pp,
            tc.tile_pool(name="rope", bufs=4) as rp,
            tc.tile_pool(name="exps", bufs=8) as ep,
            tc.tile_pool(name="norm", bufs=6) as np_,
            tc.tile_pool(name="outsb", bufs=3) as op_,
        ):
            # ---- persistent SBUF tiles + input loads (chunked for DMA spread)
            h_sb = pp.tile([128, 8, S], BF, tag="h_sb")
            wqk_sb = pp.tile([128, 8, 384], BF, tag="wqk_sb")
            wo_sb = pp.tile([128, 2, HID], BF, tag="wo_sb")
            cos_sb = pp.tile([64, S], F32, tag="cos_sb")
            sin_sb = pp.tile([64, S], F32, tag="sin_sb")
            h_dram = hT.rearrange("(c p) s -> p c s", p=128)
            wqk_dram = wqkT.rearrange("(c p) r -> p c r", p=128)
            wo_dram = woT.rearrange("(c p) h -> p c h", p=128)
            for hc in range(8):
                nc.sync.dma_start(wqk_sb[:, hc, :], wqk_dram[:, hc, :])
            for tt in range(NTT):
                for hc in range(8):
                    tts_ = bass.ts(tt, TT)
                    nc.sync.dma_start(h_sb[:, hc, tts_], h_dram[:, hc, tts_])
            nc.sync.dma_start(cos_sb[:, : S // 2], cosd[:, : S // 2])
            nc.sync.dma_start(cos_sb[:, S // 2 :], cosd[:, S // 2 :])
            nc.sync.dma_start(sin_sb[:, : S // 2], sind[:, : S // 2])
            nc.sync.dma_start(sin_sb[:, S // 2 :], sind[:, S // 2 :])
            for oc in range(2):
                nc.sync.dma_start(wo_sb[:, oc, :], wo_dram[:, oc, :])

            ident = pp.tile([64, 64], BF, tag="ident")
            make_identity(nc, ident[:])
            ones64 = pp.tile([1, 64], F32, tag="ones64")
            nc.any.memset(ones64[:], 1.0)
            # preload the exp table set while input DMAs stream
            warm = pp.tile([1, 8], F32, tag="warm")
            nc.any.memset(warm[:], 0.0)
            nc.scalar.activation(warm[:], warm[:], AF.Exp)

            # roped q, 2 heads per tile (head 2p at rows 0:64, 2p+1 at 64:128)
            qrot = [pp.tile([128, S], BF, tag=f"qrot{p}", name=f"qrot{p}") for p in range(2)]
            # roped k duplicated on both partition halves
            k2 = pp.tile([128, S], BF, tag="k2")
            vT = pp.tile([64, S], BF, tag="vT")
            # V with ones column for fused softmax denominator
            vaug = pp.tile([128, KC, 65], BF, tag="vaug")
            nc.any.memset(vaug[:], 1.0)
            # normalized attention output (o-chunk tiles), [o, t] layout
            anorm = [pp.tile([128, S], BF, tag=f"anorm{o}", name=f"anorm{o}") for o in range(2)]

            def rope64(ps_blk, dests, tts, tmp_tag):
                """RoPE one 64-row head block [64, TT] (psum) -> dest slices."""
                t1 = rp.tile([64, TT], F32, tag=f"t1{tmp_tag}")
                rt = rp.tile([64, TT], F32, tag=f"rt{tmp_tag}")
                nc.vector.tensor_tensor(t1[:], ps_blk[0:64, :], cos_sb[:, tts], MULT)
                nc.vector.tensor_tensor(
                    rt[0:32, :], ps_blk[32:64, :], sin_sb[0:32, tts], MULT
                )
                nc.vector.tensor_tensor(
                    rt[32:64, :], ps_blk[0:32, :], sin_sb[32:64, tts], MULT
                )
                nc.vector.tensor_tensor(dests[0], t1[:], rt[:], ADD)
                for dest in dests[1:]:
                    # duplicate halves via the idle gpsimd engine
                    nc.gpsimd.tensor_copy(dest, dests[0])

            # ---- single PSUM layout: proj/misc 2 banks, scores 4, acc 2 ----
            F32R = mybir.dt.float32r
            with (
                tc.tile_pool(name="psP", bufs=2, space="PSUM") as psP,
                tc.tile_pool(name="psS", bufs=2, space="PSUM") as psS,
                tc.tile_pool(name="psACC", bufs=1, space="PSUM") as psACC,
            ):

                def proj_chunk(rc, only_tt=None):
                    for tt in range(NTT):
                        if only_tt is not None and tt != only_tt:
                            continue
                        tts = bass.ts(tt, TT)
                        ps = psP.tile([128, TT], F32, tag="proj",
                                      name=f"proj{rc}_{tt}")
                        for hc in range(8):
                            nc.tensor.matmul(
                                ps[:],
                                wqk_sb[:, hc, bass.ts(rc, 128)],
                                h_sb[:, hc, tts],
                                start=(hc == 0),
                                stop=(hc == 7),
                            )
                        if rc == 2:
                            nc.vector.tensor_copy(vT[:, tts], ps[64:128, :])
                            # k rows 0:64 -> rope -> both halves of k2
                            rope64(
                                ps[0:64, :],
                                [k2[0:64, tts], k2[64:128, tts]],
                                tts,
                                "k",
                            )
                        else:
                            rope64(ps[0:64, :], [qrot[rc][0:64, tts]], tts, "qa")
                            rope64(ps[64:128, :], [qrot[rc][64:128, tts]], tts, "qb")

                def attention_qt(pair, qt):
                        qts = bass.ts(qt, TT)
                        pacc = psACC.tile([65, 2 * TT], F32, tag="att",
                                          name=f"att{pair}_{qt}")
                        for c in range(KC):
                            cs = bass.ts(c, 128)
                            # both heads' score tiles side by side -> one exp
                            sc2 = psS.tile([128, 2 * TT], F32, tag="sc",
                                           name=f"sc{pair}_{qt}_{c}")
                            nc.tensor.matmul(
                                sc2[:, 0:TT],
                                k2[0:64, cs],
                                qrot[pair][0:64, qts],
                                start=True,
                                stop=True,
                                tile_position=(0, 0),
                            )
                            nc.tensor.matmul(
                                sc2[:, TT : 2 * TT],
                                k2[64:128, cs],
                                qrot[pair][64:128, qts],
                                start=True,
                                stop=True,
                                tile_position=(64, 0),
                            )
                            ex = ep.tile([128, 2 * TT], BF, tag="exp")
                            nc.scalar.activation(ex[:], sc2[:], AF.Exp)
                            for i in range(2):
                                nc.tensor.matmul(
                                    pacc[:, bass.ts(i, TT)],
                                    vaug[:, c, :],
                                    ex[:, bass.ts(i, TT)],
                                    start=(c == 0),
                                    stop=(c == KC - 1),
                                )
                        # normalize: attn[0:64] / attn[64] per head
                        for i in range(2):
                            its = bass.ts(i, TT)
                            den = np_.tile([1, TT], F32, tag="den")
                            nc.vector.tensor_copy(den[:], pacc[64:65, its])
                            rec = np_.tile([1, TT], F32, tag="rec")
                            nc.vector.reciprocal_approx_fast(rec[:], den[:])
                            bc = psP.tile([64, TT], F32, tag="proj",
                                          name=f"bc{pair}_{qt}_{i}")
                            nc.tensor.matmul(
                                bc[:], ones64[:], rec[:], start=True, stop=True
                            )
                            bcs = np_.tile([64, TT], F32, tag="bcs")
                            nc.vector.tensor_copy(bcs[:], bc[:])
                            nc.vector.tensor_tensor(
                                anorm[pair][bass.ts(i, 64), qts],
                                pacc[0:64, its],
                                bcs[:],
                                MULT,
                            )

                def transp_tt(tt_):
                    # V^T -> V transpose (PE) into vaug as soon as vT is ready
                    for c in range(4 * tt_, 4 * tt_ + 4):
                        pt = psP.tile([128, 64], BF, tag="proj", name=f"vt{c}")
                        nc.tensor.transpose(pt[:], vT[:, bass.ts(c, 128)],
                                            ident[:])
                        nc.vector.tensor_copy(vaug[:, c, 0:64], pt[:])

                for tt_ in range(NTT):
                    proj_chunk(2, only_tt=tt_)
                    transp_tt(tt_)
                proj_chunk(0, only_tt=0)
                # interleave remaining projections with pair-0 attention
                for qt in range(NTT):
                    if qt + 1 < NTT:
                        proj_chunk(0, only_tt=qt + 1)
                    proj_chunk(1, only_tt=qt)
                    attention_qt(0, qt)
                def outproj_qt(qt):
                    # token chunks whose anorm columns are complete after
                    # both pairs finished q-tile qt; the last q-tile borrows
                    # the scores pool, free once the final exp has drained
                    pool, ptag = (psS, "sc") if qt == NTT - 1 else (psP, "proj")
                    for tch in range(4 * qt, 4 * qt + 4):
                        tcs = bass.ts(tch, 128)
                        for ht in range(2):
                            hts = bass.ts(ht, TT)
                            po = pool.tile([128, TT], F32, tag=ptag,
                                           name=f"po{tch}_{ht}")
                            for oc in range(2):
                                nc.tensor.matmul(
                                    po[:],
                                    anorm[oc][:, tcs],
                                    wo_sb[:, oc, hts],
                                    start=(oc == 0),
                                    stop=(oc == 1),
                                )
                            ob = op_.tile([128, TT], F32, tag="ob")
                            nc.vector.tensor_copy(ob[:], po[:])
                            nc.sync.dma_start(out[tcs, hts], ob[:])

                for qt in range(NTT):
                    attention_qt(1, qt)
                    outproj_qt(qt)
    nc.finalize()
    return nc


def _get_nc():
    global _nc_cache
    if _nc_cache is None:
        _nc_cache = _build_bass()
    return _nc_cache


def _shard_inputs(hidden_states, cos, sin, w_qkv, w_o):
    """Build per-core input maps. Core c = (b = c // 4, g = c % 4)."""
    cosT = np.ascontiguousarray(cos.T.astype(np.float32))          # [64, S]
    sinT = sin.T.astype(np.float32)
    sinmod = np.concatenate([-sinT[0:32], sinT[32:64]], axis=0)    # sign folded
    sinmod = np.ascontiguousarray(sinmod)

    hT = [
        np.ascontiguousarray(hidden_states[b].T).astype(_BF16) for b in range(B)
    ]
    in_maps = []
    for c in range(NCORES):
        b, g = divmod(c, 4)
        q_rows = w_qkv[256 * g : 256 * g + 256] * SCALE
        k_rows = w_qkv[1024 + 64 * g : 1024 + 64 * g + 64]
        v_rows = w_qkv[1280 + 64 * g : 1280 + 64 * g + 64]
        wqk = np.concatenate([q_rows, k_rows, v_rows], axis=0)     # [384, 1024]
        wqkT = np.ascontiguousarray(wqk.T).astype(_BF16)           # [1024, 384]
        woT = np.ascontiguousarray(
            w_o[:, 256 * g : 256 * g + 256].T
        ).astype(_BF16)                                            # [256, 1024]
        in_maps.append(
            {
                "hT": hT[b],
                "wqkT": wqkT,
                "woT": woT,
                "cosd": cosT,
                "sind": sinmod,
            }
        )
    return in_maps


def _run(inputs, **spmd_kwargs):
    from concourse.bass_utils import run_bass_kernel_spmd

    nc = _get_nc()
    in_maps = _shard_inputs(**inputs)
    res = run_bass_kernel_spmd(
        nc, in_maps, core_ids=list(range(NCORES)), **spmd_kwargs
    )
    outs = []
    for b in range(B):
        acc = res.results[4 * b]["out"].astype(np.float32).copy()
        for g in range(1, 4):
            acc += res.results[4 * b + g]["out"]
        outs.append(acc)
    return np.stack(outs, axis=0), res


def kernel(**inputs):
    out, _ = _run(inputs)
    return out


# revision 30
# speedup vs baseline: 1.0374x; 1.0374x over previous
"""GQA attention layer (QKV proj + RoPE + softmax attention + out proj) on 8
Trainium2 NeuronCores.

Sharding: core c = (batch b, head-group g) with b = c // 4, g = c % 4.
Each core handles one batch and one GQA group (4 q heads + 1 kv head),
computes a w_o-column-sliced partial output (row-parallel), and the host sums
the 4 partials per batch (the "all-reduce" of the hinted TP scheme, done on
host since the final gather happens there anyway).
"""

import numpy as np
import ml_dtypes

B, S, HID = 2, 2048, 1024
NH, NKV, D = 16, 4, 64
SCALE = D ** -0.5
NCORES = 8
TT = 512          # token tile (projection N / q tile)
NTT = S // TT     # 4
KC = S // 128     # 16 k chunks

_BF16 = ml_dtypes.bfloat16

_nc_cache = None


def _build_bass():
    import concourse.bass as bass
    import concourse.mybir as mybir
    import concourse.tile as tile
    from concourse import bacc
    from concourse.masks import make_identity

    BF = mybir.dt.bfloat16
    F32 = mybir.dt.float32
    AF = mybir.ActivationFunctionType
    MULT = mybir.AluOpType.mult
    ADD = mybir.AluOpType.add

    nc = bacc.Bacc()
    hT = nc.dram_tensor("hT", (HID, S), BF, kind="ExternalInput")
    wqkT = nc.dram_tensor("wqkT", (HID, 384), BF, kind="ExternalInput")
    woT = nc.dram_tensor("woT", (256, HID), BF, kind="ExternalInput")
    cosd = nc.dram_tensor("cosd", (64, S), F32, kind="ExternalInput")
    sind = nc.dram_tensor("sind", (64, S), F32, kind="ExternalInput")
    out = nc.dram_tensor("out", (S, HID), F32, kind="ExternalOutput")

    with tile.TileContext(nc) as tc:
        with (
            tc.tile_pool(name="persist", bufs=1) as pp,
            tc.tile_pool(name="rope", bufs=4) as rp,
            tc.tile_pool(name="exps", bufs=8) as ep,
            tc.tile_pool(name="norm", bufs=6) as np_,
            tc.tile_pool(name="outsb", bufs=6) as op_,
        ):
            # ---- persistent SBUF tiles + input loads (chunked for DMA spread)
            h_sb = pp.tile([128, 8, S], BF, tag="h_sb")
            wqk_sb = pp.tile([128, 8, 384], BF, tag="wqk_sb")
            wo_sb = pp.tile([128, 2, HID], BF, tag="wo_sb")
            cos_sb = pp.tile([64, S], F32, tag="cos_sb")
            sin_sb = pp.tile([64, S], F32, tag="sin_sb")
            h_dram = hT.rearrange("(c p) s -> p c s", p=128)
            wqk_dram = wqkT.rearrange("(c p) r -> p c r", p=128)
            wo_dram = woT.rearrange("(c p) h -> p c h", p=128)
            for hc in range(8):
                nc.sync.dma_start(wqk_sb[:, hc, :], wqk_dram[:, hc, :])
            for hc in range(8):
                nc.sync.dma_start(h_sb[:, hc, 0:TT], h_dram[:, hc, 0:TT])
            # rope tables right after the first token wave (first rope
            # needs them at ~10us; behind all hT they'd land too late)
            nc.sync.dma_start(cos_sb[:, : S // 2], cosd[:, : S // 2])
            nc.sync.dma_start(cos_sb[:, S // 2 :], cosd[:, S // 2 :])
            nc.sync.dma_start(sin_sb[:, : S // 2], sind[:, : S // 2])
            nc.sync.dma_start(sin_sb[:, S // 2 :], sind[:, S // 2 :])
            for tt in range(1, NTT):
                for hc in range(8):
                    tts_ = bass.ts(tt, TT)
                    nc.sync.dma_start(h_sb[:, hc, tts_], h_dram[:, hc, tts_])
            for oc in range(2):
                nc.sync.dma_start(wo_sb[:, oc, :], wo_dram[:, oc, :])

            ident = pp.tile([64, 64], BF, tag="ident")
            make_identity(nc, ident[:])
            ones64 = pp.tile([1, 64], F32, tag="ones64")
            nc.any.memset(ones64[:], 1.0)
            # preload the exp table set while input DMAs stream
            warm = pp.tile([1, 8], F32, tag="warm")
            nc.any.memset(warm[:], 0.0)
            nc.scalar.activation(warm[:], warm[:], AF.Exp)

            # roped q, 2 heads per tile (head 2p at rows 0:64, 2p+1 at 64:128)
            qrot = [pp.tile([128, S], BF, tag=f"qrot{p}", name=f"qrot{p}") for p in range(2)]
            # roped k duplicated on both partition halves
            k2 = pp.tile([128, S], BF, tag="k2")
            vT = pp.tile([64, S], BF, tag="vT")
            # V with ones column for fused softmax denominator
            vaug = pp.tile([128, KC, 65], BF, tag="vaug")
            nc.any.memset(vaug[:], 1.0)
            # normalized attention output (o-chunk tiles), [o, t] layout
            anorm = [pp.tile([128, S], BF, tag=f"anorm{o}", name=f"anorm{o}") for o in range(2)]

            def rope64(ps_blk, dests, tts, tmp_tag):
                """RoPE one 64-row head block [64, TT] (psum) -> dest slices."""
                t1 = rp.tile([64, TT], F32, tag=f"t1{tmp_tag}")
                rt = rp.tile([64, TT], F32, tag=f"rt{tmp_tag}")
                nc.vector.tensor_tensor(t1[:], ps_blk[0:64, :], cos_sb[:, tts], MULT)
                nc.vector.tensor_tensor(
                    rt[0:32, :], ps_blk[32:64, :], sin_sb[0:32, tts], MULT
                )
                nc.vector.tensor_tensor(
                    rt[32:64, :], ps_blk[0:32, :], sin_sb[32:64, tts], MULT
                )
                nc.vector.tensor_tensor(dests[0], t1[:], rt[:], ADD)
                for dest in dests[1:]:
                    # duplicate halves via the idle gpsimd engine
                    nc.gpsimd.tensor_copy(dest, dests[0])

            # ---- single PSUM layout: proj/misc 2 banks, scores 4, acc 2 ----
            F32R = mybir.dt.float32r
            with (
                tc.tile_pool(name="psP", bufs=2, space="PSUM") as psP,
                tc.tile_pool(name="psS", bufs=2, space="PSUM") as psS,
                tc.tile_pool(name="psACC", bufs=1, space="PSUM") as psACC,
            ):

                def proj_chunk(rc, only_tt=None):
                    for tt in range(NTT):
                        if only_tt is not None and tt != only_tt:
                            continue
                        tts = bass.ts(tt, TT)
                        ps = psP.tile([128, TT], F32, tag="proj",
                                      name=f"proj{rc}_{tt}")
                        for hc in range(8):
                            nc.tensor.matmul(
                                ps[:],
                                wqk_sb[:, hc, bass.ts(rc, 128)],
                                h_sb[:, hc, tts],
                                start=(hc == 0),
                                stop=(hc == 7),
                            )
                        if rc == 2:
                            nc.vector.tensor_copy(vT[:, tts], ps[64:128, :])
                            # k rows 0:64 -> rope -> both halves of k2
                            rope64(
                                ps[0:64, :],
                                [k2[0:64, tts], k2[64:128, tts]],
                                tts,
                                "k",
                            )
                        else:
                            rope64(ps[0:64, :], [qrot[rc][0:64, tts]], tts, "qa")
                            rope64(ps[64:128, :], [qrot[rc][64:128, tts]], tts, "qb")

                def attention_qt(pair, qt):
                        qts = bass.ts(qt, TT)
                        pacc = psACC.tile([65, 2 * TT], F32, tag="att",
                                          name=f"att{pair}_{qt}")
                        for c in range(KC):
                            cs = bass.ts(c, 128)
                            # both heads' score tiles side by side -> one exp
                            sc2 = psS.tile([128, 2 * TT], F32, tag="sc",
                                           name=f"sc{pair}_{qt}_{c}")
                            nc.tensor.matmul(
                                sc2[:, 0:TT],
                                k2[0:64, cs],
                                qrot[pair][0:64, qts],
                                start=True,
                                stop=True,
                                tile_position=(0, 0),
                            )
                            nc.tensor.matmul(
                                sc2[:, TT : 2 * TT],
                                k2[64:128, cs],
                                qrot[pair][64:128, qts],
                                start=True,
                                stop=True,
                                tile_position=(64, 0),
                            )
                            ex = ep.tile([128, 2 * TT], BF, tag="exp")
                            nc.scalar.activation(ex[:], sc2[:], AF.Exp)
                            for i in range(2):
                                nc.tensor.matmul(
                                    pacc[:, bass.ts(i, TT)],
                                    vaug[:, c, :],
                                    ex[:, bass.ts(i, TT)],
                                    start=(c == 0),
                                    stop=(c == KC - 1),
                                )
                        # normalize: attn[0:64] / attn[64] per head
                        for i in range(2):
                            its = bass.ts(i, TT)
                            den = np_.tile([1, TT], F32, tag="den")
                            nc.vector.tensor_copy(den[:], pacc[64:65, its])
                            rec = np_.tile([1, TT], F32, tag="rec")
                            nc.vector.reciprocal_approx_fast(rec[:], den[:])
                            bc = psP.tile([64, TT], F32, tag="proj",
                                          name=f"bc{pair}_{qt}_{i}")
                            nc.tensor.matmul(
                                bc[:], ones64[:], rec[:], start=True, stop=True
                            )
                            bcs = np_.tile([64, TT], F32, tag="bcs")
                            nc.vector.tensor_copy(bcs[:], bc[:])
                            nc.vector.tensor_tensor(
                                anorm[pair][bass.ts(i, 64), qts],
                                pacc[0:64, its],
                                bcs[:],
                                MULT,
                            )

                def transp_tt(tt_):
                    # V^T -> V transpose (PE) into vaug as soon as vT is ready
                    for c in range(4 * tt_, 4 * tt_ + 4):
                        pt = psP.tile([128, 64], BF, tag="proj", name=f"vt{c}")
                        nc.tensor.transpose(pt[:], vT[:, bass.ts(c, 128)],
                                            ident[:])
                        nc.vector.tensor_copy(vaug[:, c, 0:64], pt[:])

                for tt_ in range(NTT):
                    proj_chunk(2, only_tt=tt_)
                    transp_tt(tt_)
                proj_chunk(0, only_tt=0)
                # interleave remaining projections with pair-0 attention
                for qt in range(NTT):
                    if qt + 1 < NTT:
                        proj_chunk(0, only_tt=qt + 1)
                    proj_chunk(1, only_tt=qt)
                    attention_qt(0, qt)
                def outproj_qt(qt):
                    # token chunks whose anorm columns are complete after
                    # both pairs finished q-tile qt; the last q-tile borrows
                    # the scores pool, free once the final exp has drained
                    last = qt == NTT - 1
                    for tch in range(4 * qt, 4 * qt + 4):
                        tcs = bass.ts(tch, 128)
                        for ht in range(2):
                            hts = bass.ts(ht, TT)
                            # last q-tile: alternate between the two pools
                            # that are draining free (scores + proj)
                            pool, ptag = (
                                ((psS, "sc") if (tch + ht) % 2 else (psP, "proj"))
                                if last
                                else (psP, "proj")
                            )
                            po = pool.tile([128, TT], F32, tag=ptag,
                                           name=f"po{tch}_{ht}")
                            for oc in range(2):
                                nc.tensor.matmul(
                                    po[:],
                                    anorm[oc][:, tcs],
                                    wo_sb[:, oc, hts],
                                    start=(oc == 0),
                                    stop=(oc == 1),
                                )
                            ob = op_.tile([128, TT], F32, tag="ob")
                            nc.vector.tensor_copy(ob[:], po[:])
                            nc.sync.dma_start(out[tcs, hts], ob[:])

                for qt in range(NTT):
                    attention_qt(1, qt)
                    outproj_qt(qt)
    nc.finalize()
    return nc


def _get_nc():
    global _nc_cache
    if _nc_cache is None:
        _nc_cache = _build_bass()
    return _nc_cache


def _shard_inputs(hidden_states, cos, sin, w_qkv, w_o):
    """Build per-core input maps. Core c = (b = c // 4, g = c % 4)."""
    cosT = np.ascontiguousarray(cos.T.astype(np.float32))          # [64, S]
    sinT = sin.T.astype(np.float32)
    sinmod = np.concatenate([-sinT[0:32], sinT[32:64]], axis=0)    # sign folded
    sinmod = np.ascontiguousarray(sinmod)

    hT = [
        np.ascontiguousarray(hidden_states[b].T).astype(_BF16) for b in range(B)
    ]
    in_maps = []
    for c in range(NCORES):
        b, g = divmod(c, 4)
        q_rows = w_qkv[256 * g : 256 * g + 256] * SCALE
        k_rows = w_qkv[1024 + 64 * g : 1024 + 64 * g + 64]
        v_rows = w_qkv[1280 + 64 * g : 1280 + 64 * g + 64]
        wqk = np.concatenate([q_rows, k_rows, v_rows], axis=0)     # [384, 1024]
        wqkT = np.ascontiguousarray(wqk.T).astype(_BF16)           # [1024, 384]
        woT = np.ascontiguousarray(
            w_o[:, 256 * g : 256 * g + 256].T
        ).astype(_BF16)                                            # [256, 1024]
        in_maps.append(
            {
                "hT": hT[b],
                "wqkT": wqkT,
                "woT": woT,
                "cosd": cosT,
                "sind": sinmod,
            }
        )
    return in_maps


def _run(inputs, **spmd_kwargs):
    from concourse.bass_utils import run_bass_kernel_spmd

    nc = _get_nc()
    in_maps = _shard_inputs(**inputs)
    res = run_bass_kernel_spmd(
        nc, in_maps, core_ids=list(range(NCORES)), **spmd_kwargs
    )
    outs = []
    for b in range(B):
        acc = res.results[4 * b]["out"].astype(np.float32).copy()
        for g in range(1, 4):
            acc += res.results[4 * b + g]["out"]
        outs.append(acc)
    return np.stack(outs, axis=0), res


def kernel(**inputs):
    out, _ = _run(inputs)
    return out
